# revision 2
# baseline (speedup 1.0000x reference)
"""Grouped Conv1d (B=4, T=512, G=129, F=96 -> O=96, K=3, pad=1) on 8 trn2 cores.

Sharding: 129 groups = 16 full groups per core + group 128 split across all
8 cores by (batch b = core//2, T-half = core%2).  SPMD: every core runs the
identical program on its own slice.

Per (group, batch): out[o, t] = sum_k w_k[f, o].T @ x[f, t+k-1]  (3 matmuls
accumulated in fp32 PSUM).  x and w are cast to fp16 on the host: fp16 runs
the PE moving operand at full rate, halves the x DMA bytes, and keeps max
rel err ~5e-4 (accumulate stays fp32).  Bias is added fp32 on ScalarE /
VectorE (alternating) while copying PSUM -> SBUF; output stored fp16.

Layout: one flat x tile [96, 33154] covering all 16 padded (g,b) units plus
the tail-group slice lives in SBUF for the whole kernel (66 KB/partition).
DMA rides the two HWDGE rings with long per-partition lines (first pieces
small for latency, then 16 KB lines = 4x 4096B packets + 64B runt, ~2%
packet overhead instead of the 20% a 4112B line pays).  Output stores are
exactly 4096B/partition per group.  A short burst of dummy matmuls on a
memset scratch tile runs during the first-DMA latency window so the PE HAM
clock-gate is warm (2.4 GHz) when real matmuls start.
"""

from contextlib import ExitStack

import numpy as np

import concourse.bass as bass
import concourse.mybir as mybir
import concourse.tile as tile
from concourse import bacc
from concourse.bass_utils import run_bass_kernel_spmd

B, T, G, F, O, K = 4, 512, 129, 96, 96, 3
NCORES = 8
GPC = 16          # full groups per core (8*16 = 128; group 128 is split 8 ways)
TP = T + 2        # padded unit length per (g, b)
TE = T // 2       # tail-group T chunk per core
TEP = TE + 2
XC = GPC * B * TP + TEP      # x columns per core (33154)
OC = GPC * B * T + TE        # out columns per core (33024)
WC = (GPC + 1) * K * O       # weight columns per core (4896)
NDUMMY = 5                   # HAM warm-up matmuls


def build_program():
    nc = bacc.Bacc("TRN2", target_bir_lowering=False, debug=False,
                   num_devices=NCORES)

    f32 = mybir.dt.float32
    f16 = mybir.dt.float16

    xm = nc.dram_tensor("xm", [F, XC], f16, kind="ExternalInput")
    wt = nc.dram_tensor("wt", [F, WC], f16, kind="ExternalInput")
    bt = nc.dram_tensor("bt", [O, GPC + 1], f32, kind="ExternalInput")
    om = nc.dram_tensor("om", [O, OC], f16, kind="ExternalOutput")

    with ExitStack() as ctx:
        tc = ctx.enter_context(tile.TileContext(nc))
        wpool = ctx.enter_context(tc.tile_pool(name="w", bufs=1))
        opool = ctx.enter_context(tc.tile_pool(name="o", bufs=3))
        pspool = ctx.enter_context(tc.tile_pool(name="ps", bufs=8, space="PSUM"))

        x_sb = wpool.tile([F, XC], f16)
        w_sb = wpool.tile([F, WC], f16)
        b_sb = wpool.tile([O, GPC + 1], f32)
        scr = wpool.tile([F, O + T], f16)      # dummy-matmul operands

        nc.vector.memset(scr[:], 0.0)

        def ld(eng, u0, u1):                   # load x units [u0, u1)
            eng.dma_start(x_sb[:, u0 * TP:u1 * TP], xm[:, u0 * TP:u1 * TP])

        kw = K * O
        # prologue: tiny first pieces on both rings for latency, then long
        # lines; every ring keeps a load queued until its data is consumed
        nc.sync.dma_start(w_sb[:, :kw], wt[:, :kw])          # g0 weights
        ld(nc.scalar, 0, 1)                                  # g0 b0
        ld(nc.sync, 1, 2)                                    # g0 b1
        ld(nc.scalar, 2, 4)                                  # g0 b2-3
        nc.scalar.dma_start(b_sb[:], bt[:])
        ld(nc.sync, 4, 8)                                    # g1
        nc.sync.dma_start(w_sb[:, kw:], wt[:, kw:])          # g1-16 weights
        nc.gpsimd.dma_start(x_sb[:, GPC * B * TP:],          # tail-group x
                            xm[:, GPC * B * TP:])
        for s in range(7):                                   # g2-15: 16KB lines
            u0 = 8 + 8 * s
            ld(nc.scalar if s % 2 == 0 else nc.sync, u0, u0 + 8)

        # HAM warm-up: garbage matmuls with no DMA dependency keep the PE
        # busy from the preamble barrier until real data lands, so the
        # clock-gate releases (1.2 -> 2.4 GHz) as real matmuls begin
        for _ in range(NDUMMY):
            psd = pspool.tile([O, T], f32, tag="ps")
            nc.tensor.matmul(psd[:], scr[:, :O], scr[:, O:O + T],
                             start=True, stop=True)

        def main_group(i):
            pss = [pspool.tile([O, T], f32, tag="ps", name=f"ps{b}")
                   for b in range(B)]
            for k in range(K):
                for b in range(B):
                    u = 4 * i + b
                    nc.tensor.matmul(
                        pss[b][:],
                        w_sb[:, (i * K + k) * O:(i * K + k + 1) * O],
                        x_sb[:, u * TP + k:u * TP + k + T],
                        start=(k == 0),
                        stop=(k == K - 1),
                    )
            o_sb = opool.tile([O, B * T], f16, tag="o")
            for b in range(B):
                dst = o_sb[:, b * T:(b + 1) * T]
                if b % 2 == 0:
                    nc.scalar.add(dst, pss[b][:], b_sb[:, i:i + 1])
                else:
                    nc.vector.tensor_scalar_add(dst, pss[b][:],
                                                b_sb[:, i:i + 1])
            c0 = i * B * T
            if i < GPC - 1:
                eng = nc.sync if i % 2 == 0 else nc.scalar
                eng.dma_start(om[:, c0:c0 + B * T], o_sb[:])
            else:
                # last group: halves on both rings so the final drain is short
                h = B * T // 2
                nc.sync.dma_start(om[:, c0:c0 + h], o_sb[:, :h])
                nc.scalar.dma_start(om[:, c0 + h:c0 + B * T], o_sb[:, h:])

        def tail_group():
            ps = pspool.tile([O, TE], f32, tag="ps")
            xe0 = GPC * B * TP
            for k in range(K):
                nc.tensor.matmul(
                    ps[:],
                    w_sb[:, (GPC * K + k) * O:(GPC * K + k + 1) * O],
                    x_sb[:, xe0 + k:xe0 + k + TE],
                    start=(k == 0),
                    stop=(k == K - 1),
                )
            oe_sb = wpool.tile([O, TE], f16)
            nc.vector.tensor_scalar_add(oe_sb[:], ps[:],
                                        b_sb[:, GPC:GPC + 1])
            nc.gpsimd.dma_start(om[:, GPC * B * T:], oe_sb[:])

        for i in range(GPC):
            main_group(i)
            if i == 2:
                # tiny shared-group chunk: slot it in early so it doesn't
                # extend the kernel tail
                tail_group()

    nc.finalize()
    return nc


def shard_inputs(x, weight, bias):
    x = np.ascontiguousarray(x, dtype=np.float32)
    weight = np.ascontiguousarray(weight, dtype=np.float32)
    bias = np.ascontiguousarray(bias, dtype=np.float32)

    xp = np.pad(x, ((0, 0), (1, 1), (0, 0), (0, 0)))          # [B, TP, G, F]
    xt = xp.transpose(2, 3, 0, 1).astype(np.float16)          # [G, F, B, TP]
    # weight [G, O, F, K] -> [F, G, K, O]
    wtr = weight.transpose(2, 0, 3, 1).astype(np.float16)

    in_maps = []
    for c in range(NCORES):
        gs = list(range(c * GPC, (c + 1) * GPC)) + [G - 1]
        b_c, t0 = c // 2, (c % 2) * TE
        xm_c = np.empty((F, XC), np.float16)
        xm_c[:, :GPC * B * TP] = (
            xt[c * GPC:(c + 1) * GPC].transpose(1, 0, 2, 3)
            .reshape(F, GPC * B * TP))
        xm_c[:, GPC * B * TP:] = xt[G - 1, :, b_c, t0:t0 + TEP]
        in_maps.append({
            "xm": xm_c,
            "wt": np.ascontiguousarray(wtr[:, gs].reshape(F, WC)),
            "bt": np.ascontiguousarray(bias[gs].T),
            })
    return in_maps


def unshard_outputs(results):
    out = np.empty((B, T, G, O), dtype=np.float32)
    for c in range(NCORES):
        om = results[c]["om"].astype(np.float32)              # [O, OC]
        main = om[:, :GPC * B * T].reshape(O, GPC, B, T)
        out[:, :, c * GPC:(c + 1) * GPC, :] = main.transpose(2, 3, 1, 0)
        b_c, t0 = c // 2, (c % 2) * TE
        out[b_c, t0:t0 + TE, G - 1, :] = om[:, GPC * B * T:].T
    return out


def run(x, weight, bias, **run_kwargs):
    nc = build_program()
    in_maps = shard_inputs(x, weight, bias)
    res = run_bass_kernel_spmd(nc, in_maps, list(range(NCORES)), **run_kwargs)
    return unshard_outputs(res.results), res


def kernel(x, weight, bias):
    out, _ = run(x, weight, bias)
    return out


# revision 4
# speedup vs baseline: 1.0318x; 1.0318x over previous
"""Grouped Conv1d (B=4, T=512, G=129, F=96 -> O=96, K=3, pad=1) on 8 trn2 cores.

Sharding: 129 groups = 16 full groups per core + group 128 split across all
8 cores by (batch b = core//2, T-half = core%2).  SPMD: every core runs the
identical program on its own slice.

Per (group, batch): out[o, t] = sum_k w_k[f, o].T @ x[f, t+k-1]  (3 matmuls
accumulated in fp32 PSUM).  x and w are cast to fp16 on the host: fp16 runs
the PE moving operand at full rate, halves the x DMA bytes, and keeps max
rel err ~5e-4 (accumulate stays fp32).  Bias is added fp32 on ScalarE /
VectorE (alternating) while copying PSUM -> SBUF; output stored fp16.

Layout: one flat x tile [96, 33154] covering all 16 padded (g,b) units plus
the tail-group slice lives in SBUF for the whole kernel (66 KB/partition).
DMA rides the two HWDGE rings with long per-partition lines (first pieces
small for latency, then 16 KB lines = 4x 4096B packets + 64B runt, ~2%
packet overhead instead of the 20% a 4112B line pays).  Output stores are
exactly 4096B/partition per group.  A short burst of dummy matmuls on a
memset scratch tile runs during the first-DMA latency window so the PE HAM
clock-gate is warm (2.4 GHz) when real matmuls start.
"""

from contextlib import ExitStack

import numpy as np

import concourse.bass as bass
import concourse.mybir as mybir
import concourse.tile as tile
from concourse import bacc
from concourse.bass_utils import run_bass_kernel_spmd

B, T, G, F, O, K = 4, 512, 129, 96, 96, 3
NCORES = 8
GPC = 16          # full groups per core (8*16 = 128; group 128 is split 8 ways)
TP = T + 2        # padded unit length per (g, b)
TE = T // 2       # tail-group T chunk per core
TEP = TE + 2
XC = GPC * B * TP + TEP      # x columns per core (33154)
OC = GPC * B * T + TE        # out columns per core (33024)
WC = (GPC + 1) * K * O       # weight columns per core (4896)
NDUMMY = 5                   # HAM warm-up matmuls


def build_program():
    nc = bacc.Bacc("TRN2", target_bir_lowering=False, debug=False,
                   num_devices=NCORES)

    f32 = mybir.dt.float32
    f16 = mybir.dt.float16

    xm = nc.dram_tensor("xm", [F, XC], f16, kind="ExternalInput")
    wt = nc.dram_tensor("wt", [F, WC], f16, kind="ExternalInput")
    bt = nc.dram_tensor("bt", [O, GPC + 1], f32, kind="ExternalInput")
    om = nc.dram_tensor("om", [O, OC], f16, kind="ExternalOutput")

    with ExitStack() as ctx:
        tc = ctx.enter_context(tile.TileContext(nc))
        wpool = ctx.enter_context(tc.tile_pool(name="w", bufs=1))
        opool = ctx.enter_context(tc.tile_pool(name="o", bufs=3))
        pspool = ctx.enter_context(tc.tile_pool(name="ps", bufs=8, space="PSUM"))

        x_sb = wpool.tile([F, XC], f16)
        w_sb = wpool.tile([F, WC], f16)
        b_sb = wpool.tile([O, GPC + 1], f32)
        scr = wpool.tile([F, O + T], f16)      # dummy-matmul operands

        nc.vector.memset(scr[:], 0.0)

        def ldc(eng, c0, c1):                  # load x columns [c0, c1)
            eng.dma_start(x_sb[:, c0:c1], xm[:, c0:c1])

        kw = K * O
        # prologue: tiny first pieces on both rings for latency, then
        # ~4096B-per-partition stripes (the measured DMA sweet spot —
        # longer lines packetize worse, ~15 B/ns vs ~20 at 4KB)
        nc.sync.dma_start(w_sb[:, :kw], wt[:, :kw])          # g0 weights
        ldc(nc.scalar, 0, TP)                                # g0 b0
        ldc(nc.sync, TP, 2 * TP)                             # g0 b1
        ldc(nc.scalar, 2 * TP, 4 * TP)                       # g0 b2-3
        nc.scalar.dma_start(b_sb[:], bt[:])
        ldc(nc.sync, 4 * TP, 8 * TP)                         # g1
        # remaining weights in ~3KB lines interleaved with x stripes
        nc.sync.dma_start(w_sb[:, kw:1824], wt[:, kw:1824])  # g1-5 weights
        nc.gpsimd.dma_start(x_sb[:, GPC * B * TP:],          # tail-group x
                            xm[:, GPC * B * TP:])
        nc.scalar.dma_start(w_sb[:, 1824:3360], wt[:, 1824:3360])
        nc.sync.dma_start(w_sb[:, 3360:], wt[:, 3360:])
        c0 = 8 * TP
        s = 0
        while c0 < GPC * B * TP:                             # g2-15 stripes
            c1 = min(c0 + 2048, GPC * B * TP)
            if GPC * B * TP - c1 < 1024:                     # no runt stripe
                c1 = GPC * B * TP
            ldc(nc.scalar if s % 2 == 0 else nc.sync, c0, c1)
            c0 = c1
            s += 1

        # HAM warm-up: garbage matmuls with no DMA dependency keep the PE
        # busy from the preamble barrier until real data lands, so the
        # clock-gate releases (1.2 -> 2.4 GHz) as real matmuls begin
        for _ in range(NDUMMY):
            psd = pspool.tile([O, T], f32, tag="ps")
            nc.tensor.matmul(psd[:], scr[:, :O], scr[:, O:O + T],
                             start=True, stop=True)

        def main_group(i):
            pss = [pspool.tile([O, T], f32, tag="ps", name=f"ps{b}")
                   for b in range(B)]
            for k in range(K):
                for b in range(B):
                    u = 4 * i + b
                    nc.tensor.matmul(
                        pss[b][:],
                        w_sb[:, (i * K + k) * O:(i * K + k + 1) * O],
                        x_sb[:, u * TP + k:u * TP + k + T],
                        start=(k == 0),
                        stop=(k == K - 1),
                    )
            o_sb = opool.tile([O, B * T], f16, tag="o")
            for b in range(B):
                dst = o_sb[:, b * T:(b + 1) * T]
                if b % 2 == 0:
                    nc.scalar.add(dst, pss[b][:], b_sb[:, i:i + 1])
                else:
                    nc.vector.tensor_scalar_add(dst, pss[b][:],
                                                b_sb[:, i:i + 1])
            c0 = i * B * T
            if i < GPC - 1:
                eng = nc.sync if i % 2 == 0 else nc.scalar
                eng.dma_start(om[:, c0:c0 + B * T], o_sb[:])
            else:
                # last group: halves on both rings so the final drain is short
                h = B * T // 2
                nc.sync.dma_start(om[:, c0:c0 + h], o_sb[:, :h])
                nc.scalar.dma_start(om[:, c0 + h:c0 + B * T], o_sb[:, h:])

        def tail_group():
            ps = pspool.tile([O, TE], f32, tag="ps")
            xe0 = GPC * B * TP
            for k in range(K):
                nc.tensor.matmul(
                    ps[:],
                    w_sb[:, (GPC * K + k) * O:(GPC * K + k + 1) * O],
                    x_sb[:, xe0 + k:xe0 + k + TE],
                    start=(k == 0),
                    stop=(k == K - 1),
                )
            oe_sb = wpool.tile([O, TE], f16)
            nc.vector.tensor_scalar_add(oe_sb[:], ps[:],
                                        b_sb[:, GPC:GPC + 1])
            nc.gpsimd.dma_start(om[:, GPC * B * T:], oe_sb[:])

        for i in range(GPC):
            main_group(i)
            if i == 2:
                # tiny shared-group chunk: slot it in early so it doesn't
                # extend the kernel tail
                tail_group()

    nc.finalize()
    return nc


def shard_inputs(x, weight, bias):
    x = np.ascontiguousarray(x, dtype=np.float32)
    weight = np.ascontiguousarray(weight, dtype=np.float32)
    bias = np.ascontiguousarray(bias, dtype=np.float32)

    xp = np.pad(x, ((0, 0), (1, 1), (0, 0), (0, 0)))          # [B, TP, G, F]
    xt = xp.transpose(2, 3, 0, 1).astype(np.float16)          # [G, F, B, TP]
    # weight [G, O, F, K] -> [F, G, K, O]
    wtr = weight.transpose(2, 0, 3, 1).astype(np.float16)

    in_maps = []
    for c in range(NCORES):
        gs = list(range(c * GPC, (c + 1) * GPC)) + [G - 1]
        b_c, t0 = c // 2, (c % 2) * TE
        xm_c = np.empty((F, XC), np.float16)
        xm_c[:, :GPC * B * TP] = (
            xt[c * GPC:(c + 1) * GPC].transpose(1, 0, 2, 3)
            .reshape(F, GPC * B * TP))
        xm_c[:, GPC * B * TP:] = xt[G - 1, :, b_c, t0:t0 + TEP]
        in_maps.append({
            "xm": xm_c,
            "wt": np.ascontiguousarray(wtr[:, gs].reshape(F, WC)),
            "bt": np.ascontiguousarray(bias[gs].T),
            })
    return in_maps


def unshard_outputs(results):
    out = np.empty((B, T, G, O), dtype=np.float32)
    for c in range(NCORES):
        om = results[c]["om"].astype(np.float32)              # [O, OC]
        main = om[:, :GPC * B * T].reshape(O, GPC, B, T)
        out[:, :, c * GPC:(c + 1) * GPC, :] = main.transpose(2, 3, 1, 0)
        b_c, t0 = c // 2, (c % 2) * TE
        out[b_c, t0:t0 + TE, G - 1, :] = om[:, GPC * B * T:].T
    return out


def run(x, weight, bias, **run_kwargs):
    nc = build_program()
    in_maps = shard_inputs(x, weight, bias)
    res = run_bass_kernel_spmd(nc, in_maps, list(range(NCORES)), **run_kwargs)
    return unshard_outputs(res.results), res


def kernel(x, weight, bias):
    out, _ = run(x, weight, bias)
    return out


# revision 7
# speedup vs baseline: 1.2067x; 1.1695x over previous
"""Grouped Conv1d (B=4, T=512, G=129, F=96 -> O=96, K=3, pad=1) on 8 trn2 cores.

Sharding: 129 groups = 16 full groups per core + group 128 split across all
8 cores by (batch b = core//2, T-half = core%2).  SPMD: every core runs the
identical program on its own slice.

Per (group, batch): out[o, t] = sum_k w_k[f, o].T @ x[f, t+k-1]  (3 matmuls
accumulated in fp32 PSUM).  x and w are cast to fp16 on the host: fp16 runs
the PE moving operand at full rate, halves the x DMA bytes, and keeps max
rel err ~5e-4 (accumulate stays fp32).  Bias is added fp32 on ScalarE /
VectorE (alternating) while copying PSUM -> SBUF; output stored fp16.

Layout: one flat x tile [96, 33154] covering all 16 padded (g,b) units plus
the tail-group slice lives in SBUF for the whole kernel (66 KB/partition).
DMA rides the two HWDGE rings with long per-partition lines (first pieces
small for latency, then 16 KB lines = 4x 4096B packets + 64B runt, ~2%
packet overhead instead of the 20% a 4112B line pays).  Output stores are
exactly 4096B/partition per group.  A short burst of dummy matmuls on a
memset scratch tile runs during the first-DMA latency window so the PE HAM
clock-gate is warm (2.4 GHz) when real matmuls start.
"""

from contextlib import ExitStack

import numpy as np

import concourse.bass as bass
import concourse.mybir as mybir
import concourse.tile as tile
from concourse import bacc
from concourse.bass_utils import run_bass_kernel_spmd

B, T, G, F, O, K = 4, 512, 129, 96, 96, 3
NCORES = 8
GPC = 16          # full groups per core (8*16 = 128; group 128 is split 8 ways)
TP = T + 2        # padded unit length per (g, b)
TE = T // 2       # tail-group T chunk per core
TEP = TE + 2
XC = GPC * B * TP + TEP      # x columns per core (33154)
OC = GPC * B * T + TE        # out columns per core (33024)
WC = (GPC + 1) * K * O       # weight columns per core (4896)
NDUMMY = 7                   # HAM warm-up matmuls


def build_program():
    nc = bacc.Bacc("TRN2", target_bir_lowering=False, debug=False,
                   num_devices=NCORES)

    f32 = mybir.dt.float32
    f16 = mybir.dt.float16

    xm = nc.dram_tensor("xm", [F, XC], f16, kind="ExternalInput")
    wt = nc.dram_tensor("wt", [F, WC], f16, kind="ExternalInput")
    bt = nc.dram_tensor("bt", [O, GPC + 1], f32, kind="ExternalInput")
    om = nc.dram_tensor("om", [O, OC], f16, kind="ExternalOutput")

    with ExitStack() as ctx:
        tc = ctx.enter_context(tile.TileContext(nc))
        wpool = ctx.enter_context(tc.tile_pool(name="w", bufs=1))
        opool = ctx.enter_context(tc.tile_pool(name="o", bufs=3))
        pspool = ctx.enter_context(tc.tile_pool(name="ps", bufs=8, space="PSUM"))

        x_sb = wpool.tile([F, XC], f16)
        w_sb = wpool.tile([F, WC], f16)
        b_sb = wpool.tile([O, GPC + 1], f32)
        scr = wpool.tile([F, O + T], f16)      # dummy-matmul operands

        nc.vector.memset(scr[:], 0.0)

        def ldu(eng, u0, u1):                  # load x units [u0, u1)
            eng.dma_start(x_sb[:, u0 * TP:u1 * TP], xm[:, u0 * TP:u1 * TP])

        kw = K * O
        # prologue: at most 8 outstanding HWDGE DMAs (8 completion-sem
        # lanes exist; a 9th dma_start would stall its whole engine queue
        # on lane reuse).  Tiny first pieces for latency; later stripes
        # are issued from inside the compute loop, ~4KB per partition
        # (the packet sweet spot — longer lines packetize worse)
        nc.sync.dma_start(w_sb[:, :kw], wt[:, :kw])          # g0 weights
        ldu(nc.scalar, 0, 1)                                 # g0 b0
        ldu(nc.sync, 1, 2)                                   # g0 b1
        ldu(nc.scalar, 2, 4)                                 # g0 b2-3
        nc.scalar.dma_start(b_sb[:], bt[:])
        ldu(nc.sync, 4, 8)                                   # g1
        nc.sync.dma_start(w_sb[:, kw:1824], wt[:, kw:1824])  # g1-5 weights
        nc.scalar.dma_start(w_sb[:, 1824:3360], wt[:, 1824:3360])

        # HAM warm-up: garbage matmuls with no DMA dependency keep the PE
        # busy from the preamble barrier until real data lands, so the
        # clock-gate releases (1.2 -> 2.4 GHz) as real matmuls begin
        for _ in range(NDUMMY):
            psd = pspool.tile([O, T], f32, tag="ps")
            nc.tensor.matmul(psd[:], scr[:, :O], scr[:, O:O + T],
                             start=True, stop=True)

        def main_group(i):
            pss = [pspool.tile([O, T], f32, tag="ps", name=f"ps{b}")
                   for b in range(B)]
            for k in range(K):
                for b in range(B):
                    u = 4 * i + b
                    nc.tensor.matmul(
                        pss[b][:],
                        w_sb[:, (i * K + k) * O:(i * K + k + 1) * O],
                        x_sb[:, u * TP + k:u * TP + k + T],
                        start=(k == 0),
                        stop=(k == K - 1),
                    )
            o_sb = opool.tile([O, B * T], f16, tag="o")
            for b in range(B):
                dst = o_sb[:, b * T:(b + 1) * T]
                if b % 2 == 0:
                    nc.scalar.add(dst, pss[b][:], b_sb[:, i:i + 1])
                else:
                    nc.vector.tensor_scalar_add(dst, pss[b][:],
                                                b_sb[:, i:i + 1])
            c0 = i * B * T
            if i < GPC - 1:
                eng = nc.sync if i % 2 == 0 else nc.scalar
                eng.dma_start(om[:, c0:c0 + B * T], o_sb[:])
            else:
                # last group: halves on both rings so the final drain is short
                h = B * T // 2
                nc.sync.dma_start(om[:, c0:c0 + h], o_sb[:, :h])
                nc.scalar.dma_start(om[:, c0 + h:c0 + B * T], o_sb[:, h:])

        def tail_group():
            ps = pspool.tile([O, TE], f32, tag="ps")
            xe0 = GPC * B * TP
            for k in range(K):
                nc.tensor.matmul(
                    ps[:],
                    w_sb[:, (GPC * K + k) * O:(GPC * K + k + 1) * O],
                    x_sb[:, xe0 + k:xe0 + k + TE],
                    start=(k == 0),
                    stop=(k == K - 1),
                )
            oe_sb = wpool.tile([O, TE], f16)
            nc.vector.tensor_scalar_add(oe_sb[:], ps[:],
                                        b_sb[:, GPC:GPC + 1])
            nc.scalar.dma_start(om[:, GPC * B * T:], oe_sb[:])

        for i in range(GPC):
            # stream in x for group i+2 and the remaining small pieces;
            # issuing from inside the loop keeps <=3 DMAs in flight per
            # ring so no dma_start ever blocks on sem-lane reuse
            if i == 0:
                nc.sync.dma_start(w_sb[:, 3360:], wt[:, 3360:])  # g11-16+tail
            if i == 1:
                nc.scalar.dma_start(x_sb[:, GPC * B * TP:],      # tail-group x
                                    xm[:, GPC * B * TP:])
            if i + 2 < GPC:
                ldu(nc.sync if i % 2 == 0 else nc.scalar,
                    4 * (i + 2), 4 * (i + 3))
            main_group(i)
            if i == 4:
                # tiny shared-group chunk: slot it in early so it doesn't
                # extend the kernel tail
                tail_group()

    nc.finalize()
    return nc


def shard_inputs(x, weight, bias):
    x = np.ascontiguousarray(x, dtype=np.float32)
    weight = np.ascontiguousarray(weight, dtype=np.float32)
    bias = np.ascontiguousarray(bias, dtype=np.float32)

    xp = np.pad(x, ((0, 0), (1, 1), (0, 0), (0, 0)))          # [B, TP, G, F]
    xt = xp.transpose(2, 3, 0, 1).astype(np.float16)          # [G, F, B, TP]
    # weight [G, O, F, K] -> [F, G, K, O]
    wtr = weight.transpose(2, 0, 3, 1).astype(np.float16)

    in_maps = []
    for c in range(NCORES):
        gs = list(range(c * GPC, (c + 1) * GPC)) + [G - 1]
        b_c, t0 = c // 2, (c % 2) * TE
        xm_c = np.empty((F, XC), np.float16)
        xm_c[:, :GPC * B * TP] = (
            xt[c * GPC:(c + 1) * GPC].transpose(1, 0, 2, 3)
            .reshape(F, GPC * B * TP))
        xm_c[:, GPC * B * TP:] = xt[G - 1, :, b_c, t0:t0 + TEP]
        in_maps.append({
            "xm": xm_c,
            "wt": np.ascontiguousarray(wtr[:, gs].reshape(F, WC)),
            "bt": np.ascontiguousarray(bias[gs].T),
            })
    return in_maps


def unshard_outputs(results):
    out = np.empty((B, T, G, O), dtype=np.float32)
    for c in range(NCORES):
        om = results[c]["om"].astype(np.float32)              # [O, OC]
        main = om[:, :GPC * B * T].reshape(O, GPC, B, T)
        out[:, :, c * GPC:(c + 1) * GPC, :] = main.transpose(2, 3, 1, 0)
        b_c, t0 = c // 2, (c % 2) * TE
        out[b_c, t0:t0 + TE, G - 1, :] = om[:, GPC * B * T:].T
    return out


def run(x, weight, bias, **run_kwargs):
    nc = build_program()
    in_maps = shard_inputs(x, weight, bias)
    res = run_bass_kernel_spmd(nc, in_maps, list(range(NCORES)), **run_kwargs)
    return unshard_outputs(res.results), res


def kernel(x, weight, bias):
    out, _ = run(x, weight, bias)
    return out


# revision 9
# speedup vs baseline: 1.2436x; 1.0306x over previous
"""Grouped Conv1d (B=4, T=512, G=129, F=96 -> O=96, K=3, pad=1) on 8 trn2 cores.

Sharding: 129 groups = 16 full groups per core + group 128 split across all
8 cores by (batch b = core//2, T-half = core%2).  SPMD: every core runs the
identical program on its own slice.

Per (group, batch): out[o, t] = sum_k w_k[f, o].T @ x[f, t+k-1]  (3 matmuls
accumulated in fp32 PSUM).  x and w are cast to fp16 on the host: fp16 runs
the PE moving operand at full rate, halves the x DMA bytes, and keeps max
rel err ~5e-4 (accumulate stays fp32).  Bias is added fp32 on ScalarE /
VectorE (alternating) while copying PSUM -> SBUF; output stored fp16.

Layout: one flat x tile [96, 33154] covering all 16 padded (g,b) units plus
the tail-group slice lives in SBUF for the whole kernel (66 KB/partition).
DMA rides the two HWDGE rings with long per-partition lines (first pieces
small for latency, then 16 KB lines = 4x 4096B packets + 64B runt, ~2%
packet overhead instead of the 20% a 4112B line pays).  Output stores are
exactly 4096B/partition per group.  A short burst of dummy matmuls on a
memset scratch tile runs during the first-DMA latency window so the PE HAM
clock-gate is warm (2.4 GHz) when real matmuls start.
"""

from contextlib import ExitStack

import numpy as np

import concourse.bass as bass
import concourse.mybir as mybir
import concourse.tile as tile
from concourse import bacc
from concourse.bass_utils import run_bass_kernel_spmd

B, T, G, F, O, K = 4, 512, 129, 96, 96, 3
NCORES = 8
GPC = 16          # full groups per core (8*16 = 128; group 128 is split 8 ways)
TP = T + 2        # padded unit length per (g, b)
TE = T // 2       # tail-group T chunk per core
TEP = TE + 2
XC = GPC * B * TP + TEP      # x columns per core (33154)
OC = GPC * B * T + TE        # out columns per core (33024)
WC = (GPC + 1) * K * O       # weight columns per core (4896)
NDUMMY = 7                   # HAM warm-up matmuls


def build_program():
    nc = bacc.Bacc("TRN2", target_bir_lowering=False, debug=False,
                   num_devices=NCORES)

    f32 = mybir.dt.float32
    f16 = mybir.dt.float16

    xm = nc.dram_tensor("xm", [F, XC], f16, kind="ExternalInput")
    wt = nc.dram_tensor("wt", [F, WC], f16, kind="ExternalInput")
    bt = nc.dram_tensor("bt", [O, GPC + 1], f32, kind="ExternalInput")
    om = nc.dram_tensor("om", [O, OC], f16, kind="ExternalOutput")

    with ExitStack() as ctx:
        tc = ctx.enter_context(tile.TileContext(nc))
        wpool = ctx.enter_context(tc.tile_pool(name="w", bufs=1))
        opool = ctx.enter_context(tc.tile_pool(name="o", bufs=3))
        pspool = ctx.enter_context(tc.tile_pool(name="ps", bufs=8, space="PSUM"))

        x_sb = wpool.tile([F, XC], f16)
        w_sb = wpool.tile([F, WC], f16)
        b_sb = wpool.tile([O, GPC + 1], f32)
        scr = wpool.tile([F, O + T], f16)      # dummy-matmul operands

        nc.vector.memset(scr[:], 0.0)

        def ldu(eng, u0, u1):                  # load x units [u0, u1)
            eng.dma_start(x_sb[:, u0 * TP:u1 * TP], xm[:, u0 * TP:u1 * TP])

        kw = K * O
        # prologue: at most 8 outstanding HWDGE DMAs (8 completion-sem
        # lanes exist; a 9th dma_start would stall its whole engine queue
        # on lane reuse).  Tiny first pieces for latency; later stripes
        # are issued from inside the compute loop, ~4KB per partition
        # (the packet sweet spot — longer lines packetize worse)
        nc.sync.dma_start(w_sb[:, :kw], wt[:, :kw])          # g0 weights
        ldu(nc.scalar, 0, 1)                                 # g0 b0
        ldu(nc.sync, 1, 2)                                   # g0 b1
        ldu(nc.scalar, 2, 4)                                 # g0 b2-3
        nc.scalar.dma_start(b_sb[:], bt[:])
        ldu(nc.sync, 4, 8)                                   # g1
        nc.sync.dma_start(w_sb[:, kw:1824], wt[:, kw:1824])  # g1-5 weights
        nc.scalar.dma_start(w_sb[:, 1824:3360], wt[:, 1824:3360])

        # HAM warm-up: garbage matmuls with no DMA dependency keep the PE
        # busy from the preamble barrier until real data lands, so the
        # clock-gate releases (1.2 -> 2.4 GHz) as real matmuls begin
        for _ in range(NDUMMY):
            psd = pspool.tile([O, T], f32, tag="ps")
            nc.tensor.matmul(psd[:], scr[:, :O], scr[:, O:O + T],
                             start=True, stop=True)

        def main_group(i):
            # b-outer, k-inner: each batch's 3 taps run back-to-back, so
            # early batches start as soon as their own x unit lands (the
            # first 6 matmuls of group 0 need only units 0-1), and each
            # PSUM bank is ready for its bias-add after 3 matmuls
            o_sb = opool.tile([O, B * T], f16, tag="o")
            for b in range(B):
                ps = pspool.tile([O, T], f32, tag="ps", name=f"ps{b}")
                u = 4 * i + b
                for k in range(K):
                    nc.tensor.matmul(
                        ps[:],
                        w_sb[:, (i * K + k) * O:(i * K + k + 1) * O],
                        x_sb[:, u * TP + k:u * TP + k + T],
                        start=(k == 0),
                        stop=(k == K - 1),
                    )
                dst = o_sb[:, b * T:(b + 1) * T]
                if b % 2 == 0:
                    nc.scalar.add(dst, ps[:], b_sb[:, i:i + 1])
                else:
                    nc.vector.tensor_scalar_add(dst, ps[:],
                                                b_sb[:, i:i + 1])
            c0 = i * B * T
            if i < GPC - 1:
                eng = nc.sync if i % 2 == 0 else nc.scalar
                eng.dma_start(om[:, c0:c0 + B * T], o_sb[:])
            else:
                # last group: halves on both rings so the final drain is short
                h = B * T // 2
                nc.sync.dma_start(om[:, c0:c0 + h], o_sb[:, :h])
                nc.scalar.dma_start(om[:, c0 + h:c0 + B * T], o_sb[:, h:])

        def tail_group():
            ps = pspool.tile([O, TE], f32, tag="ps")
            xe0 = GPC * B * TP
            for k in range(K):
                nc.tensor.matmul(
                    ps[:],
                    w_sb[:, (GPC * K + k) * O:(GPC * K + k + 1) * O],
                    x_sb[:, xe0 + k:xe0 + k + TE],
                    start=(k == 0),
                    stop=(k == K - 1),
                )
            oe_sb = wpool.tile([O, TE], f16)
            nc.vector.tensor_scalar_add(oe_sb[:], ps[:],
                                        b_sb[:, GPC:GPC + 1])
            nc.scalar.dma_start(om[:, GPC * B * T:], oe_sb[:])

        for i in range(GPC):
            # stream in x for group i+2 and the remaining small pieces;
            # issuing from inside the loop keeps <=3 DMAs in flight per
            # ring so no dma_start ever blocks on sem-lane reuse
            if i == 0:
                ldu(nc.scalar, 8, 12)                            # g2
                nc.sync.dma_start(w_sb[:, 3360:], wt[:, 3360:])  # g11-16+tail
                ldu(nc.sync, 12, 16)                             # g3
            if i == 1:
                nc.scalar.dma_start(x_sb[:, GPC * B * TP:],      # tail-group x
                                    xm[:, GPC * B * TP:])
            if 1 <= i and i + 3 < GPC:
                ldu(nc.sync if i % 2 == 0 else nc.scalar,
                    4 * (i + 3), 4 * (i + 4))
            main_group(i)
            if i == 4:
                # tiny shared-group chunk: slot it in early so it doesn't
                # extend the kernel tail
                tail_group()

    nc.finalize()
    return nc


def shard_inputs(x, weight, bias):
    x = np.ascontiguousarray(x, dtype=np.float32)
    weight = np.ascontiguousarray(weight, dtype=np.float32)
    bias = np.ascontiguousarray(bias, dtype=np.float32)

    xp = np.pad(x, ((0, 0), (1, 1), (0, 0), (0, 0)))          # [B, TP, G, F]
    xt = xp.transpose(2, 3, 0, 1).astype(np.float16)          # [G, F, B, TP]
    # weight [G, O, F, K] -> [F, G, K, O]
    wtr = weight.transpose(2, 0, 3, 1).astype(np.float16)

    in_maps = []
    for c in range(NCORES):
        gs = list(range(c * GPC, (c + 1) * GPC)) + [G - 1]
        b_c, t0 = c // 2, (c % 2) * TE
        xm_c = np.empty((F, XC), np.float16)
        xm_c[:, :GPC * B * TP] = (
            xt[c * GPC:(c + 1) * GPC].transpose(1, 0, 2, 3)
            .reshape(F, GPC * B * TP))
        xm_c[:, GPC * B * TP:] = xt[G - 1, :, b_c, t0:t0 + TEP]
        in_maps.append({
            "xm": xm_c,
            "wt": np.ascontiguousarray(wtr[:, gs].reshape(F, WC)),
            "bt": np.ascontiguousarray(bias[gs].T),
            })
    return in_maps


def unshard_outputs(results):
    out = np.empty((B, T, G, O), dtype=np.float32)
    for c in range(NCORES):
        om = results[c]["om"].astype(np.float32)              # [O, OC]
        main = om[:, :GPC * B * T].reshape(O, GPC, B, T)
        out[:, :, c * GPC:(c + 1) * GPC, :] = main.transpose(2, 3, 1, 0)
        b_c, t0 = c // 2, (c % 2) * TE
        out[b_c, t0:t0 + TE, G - 1, :] = om[:, GPC * B * T:].T
    return out


def run(x, weight, bias, **run_kwargs):
    nc = build_program()
    in_maps = shard_inputs(x, weight, bias)
    res = run_bass_kernel_spmd(nc, in_maps, list(range(NCORES)), **run_kwargs)
    return unshard_outputs(res.results), res


def kernel(x, weight, bias):
    out, _ = run(x, weight, bias)
    return out
